# revision 6
# baseline (speedup 1.0000x reference)
"""BlockSparseLinear hybrid fp8/bf16 kernel for Trainium2 (8 NeuronCores).

y = x @ W.T + bias, x [8192,4096] f32, W [4096,4096] f32 (50% of 16x16
blocks zeroed), bias [4096]; 8-way data-parallel over tokens.

v2 over the 411us baseline: the fp8/bf16 split is chosen per
(out-tile, k-tile) 128x128 tile instead of per k-column. Each tile's
fp8 quantization noise is proportional to its nonzero-block mass, which
varies (Binomial over the 16x16 block mask), so a greedy pick of the
lightest tiles fits ~324 fp8 tiles into the same 2e-2 error budget vs
256 for the fixed split (predicted 1.967e-2). fp8 pairs run as
DoubleRow matmuls with stepped rhs APs (arbitrary k pairs); weights are
packed per-out-tile on the host.

Schedule: out-tiles process in descending fp8-count order. The first 7
out-tiles' window-0 DR groups are hoisted to the front (PSUM banks 1-7,
low-k pairs before high-k) so the PE has fp8 work while the 8MB bf16 x
copy streams; then phase 1 closes those 7 groups with bf16 (window 0
only — window-1 x is still in flight), phase 2 runs their window 1,
phase 3 the remaining 25 out-tiles. Input DMAs issue from the sync
sequencer in priority order (the ~650ns/trigger pacing keeps rings from
flooding); wb/w8 prefetch triggers are paced one per eviction. The last
out-tile's final window runs as two 256-wide quarters with
partition-split stores.
"""

import os

import numpy as np

N_CORES = 8
TOK = 8192
T_PER_CORE = TOK // N_CORES  # 1024
D_IN = 4096
D_OUT = 4096
P = 128
KO = D_IN // P  # 32
OC = D_OUT // P  # 32
T_FREE = 512
SX = 32.0
SW = 1024.0
S_OUT = 1.0 / (SX * SW)
ERR_TARGET = 0.0197
HOIST = 7

LAST_EXEC_NS = None

_cache = {}


def _select_tiles(x, w):
    """Greedy per-(oc,k) fp8 tile selection under the global error budget.

    Error model: independent per-element quantization noise,
    err^2(tile) = sum_{j,i in tile} (wq-w)_ij^2 * ||xq_i||^2
                                   + w_ij^2 * ||(xq-x)_i||^2,
    validated against exact simulation to ~3e-5 rel err. Budget is
    relative to ||y||^2 ~= sum_i ||x_i||^2 ||w_:i||^2 (0.9996 exact).
    Returns sel [OC,KO] bool with even per-oc counts.
    """
    import heapq

    import ml_dtypes

    E4 = ml_dtypes.float8_e4m3
    BF = ml_dtypes.bfloat16

    x8d = ((x * SX).astype(E4).astype(np.float32)) / SX
    w8d = ((w * SW).astype(E4).astype(np.float32)) / SW
    xbd = ((x * SX).astype(BF).astype(np.float32)) / SX
    wbd = ((w * SW).astype(BF).astype(np.float32)) / SW

    S_x8 = (x8d**2).sum(0)
    S_xr8 = ((x8d - x) ** 2).sum(0)
    S_xb = (xbd**2).sum(0)
    S_xrb = ((xbd - x) ** 2).sum(0)
    W2 = w**2
    E8 = ((w8d - w) ** 2 * S_x8[None, :] + W2 * S_xr8[None, :]).reshape(
        OC, P, KO, P
    ).sum(axis=(1, 3))
    Eb = ((wbd - w) ** 2 * S_xb[None, :] + W2 * S_xrb[None, :]).reshape(
        OC, P, KO, P
    ).sum(axis=(1, 3))

    den = float((x**2).sum(0) @ W2.sum(0))
    budget = ERR_TARGET**2 * den - float(Eb.sum())
    delta = E8 - Eb
    order = np.argsort(delta, axis=1)

    sel = np.zeros((OC, KO), bool)
    heap = []
    for oc in range(OC):
        c = float(delta[oc, order[oc, 0]] + delta[oc, order[oc, 1]])
        heapq.heappush(heap, (c, oc, 0))
    spent = 0.0
    while heap:
        cost, oc, idx = heapq.heappop(heap)
        if spent + cost > budget:
            break
        spent += cost
        sel[oc, order[oc, idx]] = True
        sel[oc, order[oc, idx + 1]] = True
        if idx + 3 < KO:
            c = float(delta[oc, order[oc, idx + 2]] + delta[oc, order[oc, idx + 3]])
            heapq.heappush(heap, (c, oc, idx + 2))
    return sel


def _make_plan(sel):
    ns = sel.sum(axis=1).astype(int)  # [OC], even counts
    ks8 = [np.flatnonzero(sel[oc]).tolist() for oc in range(OC)]
    ksb = [np.flatnonzero(~sel[oc]).tolist() for oc in range(OC)]
    o8 = np.concatenate([[0], np.cumsum(ns)[:-1]]).astype(int).tolist()
    ob = np.concatenate([[0], np.cumsum(KO - ns)[:-1]]).astype(int).tolist()
    perm = np.argsort(-ns, kind="stable").astype(int).tolist()
    return {
        "ns": ns.tolist(),
        "ks8": ks8,
        "ksb": ksb,
        "o8": o8,
        "ob": ob,
        "perm": perm,
        "n8t": int(ns.sum()),
        "nbt": int((KO - ns).sum()),
        "n8max": int(ns.max()),
        "nbmax": int((KO - ns).max()),
    }


def _bridged_runs(ks_list, gap):
    runs = []
    for k in sorted(ks_list):
        if runs and k - runs[-1][1] <= gap:
            runs[-1][1] = k + 1
        else:
            runs.append([k, k + 1])
    return runs


def _build_bass(plan):
    import concourse.bacc as bacc
    import concourse.mybir as mybir
    import concourse.tile as tile

    f32 = mybir.dt.float32
    f8 = mybir.dt.float8e4
    bf16 = mybir.dt.bfloat16
    DR = mybir.MatmulPerfMode.DoubleRow

    ns, ks8, ksb = plan["ns"], plan["ks8"], plan["ksb"]
    o8, ob, perm = plan["o8"], plan["ob"], plan["perm"]
    n8max, nbmax = plan["n8max"], plan["nbmax"]

    nc = bacc.Bacc(
        "TRN2",
        target_bir_lowering=False,
        debug=False,
        num_devices=N_CORES,
        name="block_sparse_linear_v2",
        dynamic_dma_scratch_size=4096,
    )

    xt8 = nc.dram_tensor("xt8", [P, KO, T_PER_CORE], f8, kind="ExternalInput")
    xtb = nc.dram_tensor("xtb", [P, KO, T_PER_CORE], bf16, kind="ExternalInput")
    wp8 = nc.dram_tensor("wp8", [P, plan["n8t"], P], f8, kind="ExternalInput")
    wpb = nc.dram_tensor("wpb", [P, plan["nbt"], P], bf16, kind="ExternalInput")
    bs = nc.dram_tensor("bs", [P, OC], f32, kind="ExternalInput")
    yt = nc.dram_tensor("yt", [OC, P, T_PER_CORE], f32, kind="ExternalOutput")

    with tile.TileContext(nc) as tc:
        with (
            tc.tile_pool(name="xpool", bufs=1) as xpool,
            tc.tile_pool(name="w8pool", bufs=8) as w8pool,
            tc.tile_pool(name="wbpool", bufs=8) as wbpool,
            tc.tile_pool(name="opool", bufs=2) as opool,
            tc.tile_pool(name="pspool", bufs=8, space="PSUM") as pspool,
        ):
            x8_sb = xpool.tile([P, KO, T_PER_CORE], f8)
            xb_sb = xpool.tile([P, KO, T_PER_CORE], bf16)
            bias_sb = xpool.tile([P, OC], f32)

            w8_tiles = {}
            wb_tiles = {}

            def w8_dma(i):
                if i >= OC:
                    return
                n = ns[perm[i]]
                if not n:
                    return
                t = w8pool.tile([P, n8max, P], f8, tag="w8", name=f"w8_{i}")
                nc.sync.dma_start(
                    t[:, :n, :], wp8[:, o8[perm[i]] : o8[perm[i]] + n, :]
                )
                w8_tiles[i] = t

            def wb_dma(i):
                if i >= OC:
                    return
                m = KO - ns[perm[i]]
                if not m:
                    return
                t = wbpool.tile([P, nbmax, P], bf16, tag="wb", name=f"wb_{i}")
                h = (m + 1) // 2
                nc.scalar.dma_start(
                    t[:, :h, :], wpb[:, ob[perm[i]] : ob[perm[i]] + h, :]
                )
                nc.scalar.dma_start(
                    t[:, h:m, :], wpb[:, ob[perm[i]] + h : ob[perm[i]] + m, :]
                )
                wb_tiles[i] = t

            wb_cursor = [2]
            w8_cursor = [HOIST]

            def prefetch_after_evict():
                if wb_cursor[0] < OC:
                    wb_dma(wb_cursor[0])
                    wb_cursor[0] += 1
                if w8_cursor[0] < OC:
                    w8_dma(w8_cursor[0])
                    w8_cursor[0] += 1

            def xb_dma_runs(win, runs):
                lo = win * T_FREE
                for a, b in runs:
                    nc.sync.dma_start(
                        xb_sb[:, a:b, lo : lo + T_FREE], xtb[:, a:b, lo : lo + T_FREE]
                    )

            def xb_split(first_ks):
                first_runs = _bridged_runs(first_ks, gap=2)
                covered = {k for a, b in first_runs for k in range(a, b)}
                rest_runs = _bridged_runs(
                    [k for k in range(KO) if k not in covered], gap=0
                )
                return first_runs, rest_runs

            # PE warmup: junk DR matmuls with no DMA dependency lift the HAM
            # clock gate while the first weight/x tiles land.
            warm_sb = xpool.tile([P, 2, T_FREE], f8)
            warm_ps = pspool.tile([P, T_FREE], f32, tag="ps", name="warm")
            nc.any.memset(warm_sb[:], 0.0)
            for _ in range(12):
                nc.tensor.matmul(
                    warm_ps[:],
                    warm_sb[:, :, 0:P],
                    warm_sb[:],
                    start=True,
                    stop=True,
                    perf_mode=DR,
                )

            # Startup DMA, all inputs on sync in priority order (the
            # per-trigger sequencer cost paces the rings): x8 window 0 and
            # the hoisted out-tiles' fp8 weights first, then bf16 x window 0
            # (first out-tile's k-tiles leading), x8 window 1, bf16 x
            # window 1. wb weights go on scalar (wb0/wb1 up front, the rest
            # paced behind evictions).
            w8_dma(0)
            nc.sync.dma_start(
                x8_sb[:, : KO // 2, 0:T_FREE], xt8[:, : KO // 2, 0:T_FREE]
            )
            w8_dma(1)
            nc.sync.dma_start(
                x8_sb[:, KO // 2 :, 0:T_FREE], xt8[:, KO // 2 :, 0:T_FREE]
            )
            for i in range(2, HOIST):
                w8_dma(i)
            nc.sync.dma_start(bias_sb[:], bs[:])
            wb_dma(0)
            wb_dma(1)
            f0, r0 = xb_split(ksb[perm[0]])
            xb_dma_runs(0, f0)
            xb_dma_runs(0, r0)
            nc.sync.dma_start(
                x8_sb[:, : KO // 2, T_FREE:], xt8[:, : KO // 2, T_FREE:]
            )
            nc.sync.dma_start(
                x8_sb[:, KO // 2 :, T_FREE:], xt8[:, KO // 2 :, T_FREE:]
            )
            f1, r1 = xb_split(ksb[perm[0]])
            xb_dma_runs(1, f1)
            xb_dma_runs(1, r1)

            def dr_pairs(oc):
                ks = ks8[oc]
                return [(ks[2 * j], ks[2 * j + 1]) for j in range(ns[oc] // 2)]

            def dr_mm(i, ps, lo, width, oc, j2, start):
                k1, k2 = ks8[oc][2 * j2], ks8[oc][2 * j2 + 1]
                nc.tensor.matmul(
                    ps[:],
                    w8_tiles[i][:, 2 * j2 : 2 * j2 + 2, :],
                    x8_sb[:, k1 : k2 + 1 : (k2 - k1), lo : lo + width],
                    start=start,
                    stop=False,
                    perf_mode=DR,
                )

            def dr_mms(i, ps, lo, width, oc):
                for j2 in range(ns[oc] // 2):
                    dr_mm(i, ps, lo, width, oc, j2, start=(j2 == 0))

            def bf_mms(i, ps, lo, width, oc):
                wbt = wb_tiles.get(i)
                m = KO - ns[oc]
                for j in range(m):
                    k = ksb[oc][j]
                    nc.tensor.matmul(
                        ps[:],
                        wbt[:, j, :],
                        xb_sb[:, k, lo : lo + width],
                        start=(ns[oc] == 0 and j == 0),
                        stop=(j == m - 1),
                    )

            # Hoisted DR groups: out-tiles 0..HOIST-1, window 0, on PSUM
            # banks 1..7 (bank 0 is the warmup's, freed immediately).
            # Low-k pairs for all tiles first so the PE doesn't wait on the
            # second half of x8 window 0.
            ps_handles = {}
            for i in range(HOIST):
                ps_handles[(i, 0)] = pspool.tile(
                    [P, T_FREE], f32, tag="ps", name=f"ps_{i}_0"
                )
            started = set()
            for phase in (0, 1):
                for i in range(HOIST):
                    oc = perm[i]
                    for j2, (k1, k2) in enumerate(dr_pairs(oc)):
                        if (k2 < KO // 2) == (phase == 0):
                            dr_mm(
                                i,
                                ps_handles[(i, 0)],
                                0,
                                T_FREE,
                                oc,
                                j2,
                                start=(i not in started) and not started.add(i),
                            )

            def evict(i, oc, psv, lo, width, split):
                o_sb = opool.tile([P, T_FREE], f32, tag="o", name=f"o_{i}_{lo}")
                nc.scalar.activation(
                    o_sb[:, 0:width],
                    psv,
                    mybir.ActivationFunctionType.Identity,
                    bias=bias_sb[:, oc : oc + 1],
                    scale=S_OUT,
                )
                if not split:
                    nc.scalar.dma_start(
                        yt[oc, :, lo : lo + width], o_sb[:, 0:width]
                    )
                else:
                    pq = P // 2
                    for q in range(2):
                        eng = nc.sync if q == 1 else nc.scalar
                        eng.dma_start(
                            yt[oc, q * pq : (q + 1) * pq, lo : lo + width],
                            o_sb[q * pq : (q + 1) * pq, 0:width],
                        )
                prefetch_after_evict()

            def process(i, win):
                oc = perm[i]
                last = i == OC - 1
                if last and win == 1:
                    windows = [(T_FREE, 256), (T_FREE + 256, 256)]
                else:
                    windows = [(win * T_FREE, T_FREE)]
                for lo, width in windows:
                    ps = ps_handles.pop((i, win), None)
                    if ps is None:
                        ps = pspool.tile(
                            [P, T_FREE], f32, tag="ps", name=f"ps_{i}_{lo}"
                        )
                        dr_mms(i, ps[:, 0:width], lo, width, oc)
                    bf_mms(i, ps[:, 0:width], lo, width, oc)
                    evict(i, oc, ps[:, 0:width], lo, width, split=last)

            # Phase 1: close the hoisted window-0 groups with bf16.
            for i in range(HOIST):
                process(i, 0)
            # Phase 2: the hoisted out-tiles' window 1.
            for i in range(HOIST):
                process(i, 1)
            # Phase 3: remaining out-tiles, both windows.
            for i in range(HOIST, OC):
                process(i, 0)
                process(i, 1)

    nc.compile()
    return nc


def _pack_inputs(x, weight, bias, plan):
    import ml_dtypes

    E4 = ml_dtypes.float8_e4m3
    BF = ml_dtypes.bfloat16

    x8 = (x * SX).astype(E4)
    xb = (x * SX).astype(BF)
    w8 = (weight * SW).astype(E4).reshape(OC, P, KO, P)
    wb = (weight * SW).astype(BF).reshape(OC, P, KO, P)

    xt8 = np.ascontiguousarray(
        x8.reshape(N_CORES, T_PER_CORE, KO, P).transpose(0, 3, 2, 1)
    )
    xtb = np.ascontiguousarray(
        xb.reshape(N_CORES, T_PER_CORE, KO, P).transpose(0, 3, 2, 1)
    )
    wp8 = np.ascontiguousarray(
        np.concatenate(
            [w8[oc][:, plan["ks8"][oc], :].transpose(2, 1, 0) for oc in range(OC)],
            axis=1,
        )
    )
    wpb = np.ascontiguousarray(
        np.concatenate(
            [wb[oc][:, plan["ksb"][oc], :].transpose(2, 1, 0) for oc in range(OC)],
            axis=1,
        )
    )
    bsr = np.ascontiguousarray(bias.reshape(OC, P).T)
    return xt8, xtb, wp8, wpb, bsr


def kernel(x, weight, bias):
    global LAST_EXEC_NS
    from concourse import bass_utils

    x = np.ascontiguousarray(x, dtype=np.float32)
    weight = np.ascontiguousarray(weight, dtype=np.float32)
    bias = np.ascontiguousarray(bias, dtype=np.float32)

    if "nc" not in _cache:
        sel = _select_tiles(x, weight)
        plan = _make_plan(sel)
        _cache["plan"] = plan
        _cache["nc"] = _build_bass(plan)
    nc = _cache["nc"]
    plan = _cache["plan"]

    xt8, xtb, wp8, wpb, bsr = _pack_inputs(x, weight, bias, plan)

    in_maps = [
        {"xt8": xt8[c], "xtb": xtb[c], "wp8": wp8, "wpb": wpb, "bs": bsr}
        for c in range(N_CORES)
    ]

    trace = bool(int(os.environ.get("BSL_TRACE", "0")))
    kw = {}
    if os.environ.get("BSL_TMPDIR"):
        kw["tmpdir"] = os.environ["BSL_TMPDIR"]
    res = bass_utils.run_bass_kernel_spmd(
        nc,
        in_maps,
        core_ids=list(range(N_CORES)),
        trace=trace,
        **kw,
    )
    _cache["res"] = res
    LAST_EXEC_NS = res.exec_time_ns

    out = np.empty((TOK, D_OUT), dtype=np.float32)
    for c in range(N_CORES):
        yt_out = res.results[c]["yt"]
        out[c * T_PER_CORE : (c + 1) * T_PER_CORE] = (
            yt_out.transpose(2, 0, 1).reshape(T_PER_CORE, D_OUT)
        )
    return out


# revision 8
# speedup vs baseline: 1.0304x; 1.0304x over previous
"""BlockSparseLinear hybrid fp8/bf16 kernel for Trainium2 (8 NeuronCores).

y = x @ W.T + bias, x [8192,4096] f32, W [4096,4096] f32 (50% of 16x16
blocks zeroed), bias [4096]; 8-way data-parallel over tokens.

v3 over the 411us baseline (v2 lost its PE-work win to a DMA-bound
head, measured 417us):
- fp8/bf16 split chosen per (out-tile, k-tile) 128x128 tile: greedy
  pick of lightest-noise tiles fits ~324 fp8 tiles in the 2e-2 budget
  vs 256 for the per-k split (predicted 1.97e-2). fp8 pairs run as
  DoubleRow matmuls with stepped rhs APs; weights packed per-out-tile.
- The 8MB bf16 x copy is NOT DMA'd: it is reconstructed on the idle
  DVE as bf16(x8 + xr8) from the fp8 x and an fp8 residual stream
  (4MB), cutting input x traffic from 12MB to 8MB. Reconstruction is
  as accurate as direct bf16 quantization.
- x streams are packed window-major in DRAM so every transfer is
  contiguous per partition (v2's 512B strided runs tanked DMA
  efficiency).
- Startup: the 7 fp8-heaviest out-tiles' window-0 DR groups are
  hoisted to the front (PSUM banks 1-7) covering the x-residual
  stream; phase 1 closes them with bf16, phase 2 runs their window 1,
  phase 3 the rest in ascending fp8-count order (cheapest tail last).
  Input DMAs issue from sync in priority order; wb streams from scalar
  paced one per eviction; w8 prefetch deferred to phase 2.
- Steady windows run bf16 first, fp8 pairs last.
- Last out-tile's final window: two 256-wide quarters,
  partition-split stores.
"""

import os

import numpy as np

N_CORES = 8
TOK = 8192
T_PER_CORE = TOK // N_CORES  # 1024
D_IN = 4096
D_OUT = 4096
P = 128
KO = D_IN // P  # 32
OC = D_OUT // P  # 32
NT = 2
T_FREE = 512
SX = 32.0
SW = 1024.0
S_OUT = 1.0 / (SX * SW)
ERR_TARGET = 0.0197
HOIST = 7
XR_CHUNK = 4  # max k-tiles per x-residual DMA/DVE-add chunk

LAST_EXEC_NS = None

_cache = {}


def _quant_arrays(x):
    import ml_dtypes

    E4 = ml_dtypes.float8_e4m3
    BF = ml_dtypes.bfloat16
    xs = x * SX
    x8 = xs.astype(E4)
    x8d = x8.astype(np.float32)
    xr8 = (xs - x8d).astype(E4)
    recon = (x8d + xr8.astype(np.float32)).astype(BF)
    return x8, xr8, x8d / SX, recon.astype(np.float32) / SX


def _select_tiles(x, w, x8d, xbd):
    """Greedy per-(oc,k) fp8 tile selection under the global error budget.

    Error model: independent per-element quantization noise,
    err^2(tile) = sum_{j,i in tile} (wq-w)_ij^2 * ||xq_i||^2
                                   + w_ij^2 * ||(xq-x)_i||^2,
    validated against exact simulation to ~3e-5 rel err. Budget is
    relative to ||y||^2 ~= sum_i ||x_i||^2 ||w_:i||^2 (0.9996 exact).
    xbd is the DVE-reconstructed bf16 x (dequantized). Returns sel
    [OC,KO] bool with even per-oc counts.
    """
    import heapq

    import ml_dtypes

    E4 = ml_dtypes.float8_e4m3
    BF = ml_dtypes.bfloat16

    w8d = ((w * SW).astype(E4).astype(np.float32)) / SW
    wbd = ((w * SW).astype(BF).astype(np.float32)) / SW

    S_x8 = (x8d**2).sum(0)
    S_xr8 = ((x8d - x) ** 2).sum(0)
    S_xb = (xbd**2).sum(0)
    S_xrb = ((xbd - x) ** 2).sum(0)
    W2 = w**2
    E8 = ((w8d - w) ** 2 * S_x8[None, :] + W2 * S_xr8[None, :]).reshape(
        OC, P, KO, P
    ).sum(axis=(1, 3))
    Eb = ((wbd - w) ** 2 * S_xb[None, :] + W2 * S_xrb[None, :]).reshape(
        OC, P, KO, P
    ).sum(axis=(1, 3))

    den = float((x**2).sum(0) @ W2.sum(0))
    budget = ERR_TARGET**2 * den - float(Eb.sum())
    delta = E8 - Eb
    order = np.argsort(delta, axis=1)

    sel = np.zeros((OC, KO), bool)
    heap = []
    for oc in range(OC):
        c = float(delta[oc, order[oc, 0]] + delta[oc, order[oc, 1]])
        heapq.heappush(heap, (c, oc, 0))
    spent = 0.0
    while heap:
        cost, oc, idx = heapq.heappop(heap)
        if spent + cost > budget:
            break
        spent += cost
        sel[oc, order[oc, idx]] = True
        sel[oc, order[oc, idx + 1]] = True
        if idx + 3 < KO:
            c = float(delta[oc, order[oc, idx + 2]] + delta[oc, order[oc, idx + 3]])
            heapq.heappush(heap, (c, oc, idx + 2))
    return sel


def _make_plan(sel):
    ns = sel.sum(axis=1).astype(int)  # [OC], even counts
    ks8 = [np.flatnonzero(sel[oc]).tolist() for oc in range(OC)]
    ksb = [np.flatnonzero(~sel[oc]).tolist() for oc in range(OC)]
    o8 = np.concatenate([[0], np.cumsum(ns)[:-1]]).astype(int).tolist()
    ob = np.concatenate([[0], np.cumsum(KO - ns)[:-1]]).astype(int).tolist()
    # hoisted: 7 largest fp8 counts (descending); rest ascending so the
    # final out-tile processed has the least bf16 work (short tail)
    desc = np.argsort(-ns, kind="stable")
    perm = desc[:HOIST].tolist() + desc[HOIST:][::-1].tolist()
    return {
        "ns": ns.tolist(),
        "ks8": ks8,
        "ksb": ksb,
        "o8": o8,
        "ob": ob,
        "perm": perm,
        "n8t": int(ns.sum()),
        "nbt": int((KO - ns).sum()),
        "n8max": int(ns.max()),
        "nbmax": int((KO - ns).max()),
    }


def _chunk_runs(ks_list, gap, chunk):
    """Bridged runs (merge gaps <= gap) split into <= chunk-sized pieces."""
    runs = []
    for k in sorted(ks_list):
        if runs and k - runs[-1][1] <= gap:
            runs[-1][1] = k + 1
        else:
            runs.append([k, k + 1])
    out = []
    for a, b in runs:
        while b - a > chunk:
            out.append((a, a + chunk))
            a += chunk
        out.append((a, b))
    return out


def _build_bass(plan):
    import concourse.bacc as bacc
    import concourse.mybir as mybir
    import concourse.tile as tile

    f32 = mybir.dt.float32
    f8 = mybir.dt.float8e4
    bf16 = mybir.dt.bfloat16
    DR = mybir.MatmulPerfMode.DoubleRow

    ns, ks8, ksb = plan["ns"], plan["ks8"], plan["ksb"]
    o8, ob, perm = plan["o8"], plan["ob"], plan["perm"]
    n8max, nbmax = plan["n8max"], plan["nbmax"]

    nc = bacc.Bacc(
        "TRN2",
        target_bir_lowering=False,
        debug=False,
        num_devices=N_CORES,
        name="block_sparse_linear_v3",
        dynamic_dma_scratch_size=4096,
    )

    xt8 = nc.dram_tensor("xt8", [NT, P, KO, T_FREE], f8, kind="ExternalInput")
    xtr = nc.dram_tensor("xtr", [NT, P, KO, T_FREE], f8, kind="ExternalInput")
    wp8 = nc.dram_tensor("wp8", [P, plan["n8t"], P], f8, kind="ExternalInput")
    wpb = nc.dram_tensor("wpb", [P, plan["nbt"], P], bf16, kind="ExternalInput")
    bs = nc.dram_tensor("bs", [P, OC], f32, kind="ExternalInput")
    yt = nc.dram_tensor("yt", [OC, P, T_PER_CORE], f32, kind="ExternalOutput")

    with tile.TileContext(nc) as tc:
        with (
            tc.tile_pool(name="xpool", bufs=1) as xpool,
            tc.tile_pool(name="xrpool", bufs=4) as xrpool,
            tc.tile_pool(name="w8pool", bufs=8) as w8pool,
            tc.tile_pool(name="wbpool", bufs=8) as wbpool,
            tc.tile_pool(name="opool", bufs=3) as opool,
            tc.tile_pool(name="pspool", bufs=8, space="PSUM") as pspool,
        ):
            x8_sb = xpool.tile([P, NT, KO, T_FREE], f8)
            xb_sb = xpool.tile([P, NT, KO, T_FREE], bf16)
            bias_sb = xpool.tile([P, OC], f32)

            w8_tiles = {}
            wb_tiles = {}

            def w8_dma(i):
                if i >= OC:
                    return
                n = ns[perm[i]]
                if not n:
                    return
                t = w8pool.tile([P, n8max, P], f8, tag="w8", name=f"w8_{i}")
                nc.sync.dma_start(
                    t[:, :n, :], wp8[:, o8[perm[i]] : o8[perm[i]] + n, :]
                )
                w8_tiles[i] = t

            def wb_dma(i):
                if i >= OC:
                    return
                m = KO - ns[perm[i]]
                if not m:
                    return
                t = wbpool.tile([P, nbmax, P], bf16, tag="wb", name=f"wb_{i}")
                h = (m + 1) // 2
                nc.scalar.dma_start(
                    t[:, :h, :], wpb[:, ob[perm[i]] : ob[perm[i]] + h, :]
                )
                nc.scalar.dma_start(
                    t[:, h:m, :], wpb[:, ob[perm[i]] + h : ob[perm[i]] + m, :]
                )
                wb_tiles[i] = t

            wb_cursor = [2]
            w8_cursor = [HOIST]
            w8_gate = [False]

            def prefetch_after_evict():
                if wb_cursor[0] < OC:
                    wb_dma(wb_cursor[0])
                    wb_cursor[0] += 1
                if w8_gate[0] and w8_cursor[0] < OC:
                    w8_dma(w8_cursor[0])
                    w8_cursor[0] += 1

            def xr_stream(win, chunks):
                # fp8 residual chunk in, DVE add x8+xr8 -> bf16 window of xb
                for a, b in chunks:
                    st = xrpool.tile(
                        [P, XR_CHUNK, T_FREE], f8, tag="xr", name=f"xr_{win}_{a}"
                    )
                    nc.sync.dma_start(
                        st[:, : b - a, :], xtr[win, :, a:b, :]
                    )
                    nc.vector.tensor_tensor(
                        xb_sb[:, win, a:b, :],
                        x8_sb[:, win, a:b, :],
                        st[:, : b - a, :],
                        mybir.AluOpType.add,
                    )

            # PE warmup: junk DR matmuls with no DMA dependency lift the HAM
            # clock gate while the first weight/x tiles land.
            warm_sb = xpool.tile([P, 2, T_FREE], f8)
            warm_ps = pspool.tile([P, T_FREE], f32, tag="ps", name="warm")
            nc.any.memset(warm_sb[:], 0.0)
            for _ in range(12):
                nc.tensor.matmul(
                    warm_ps[:],
                    warm_sb[:, :, 0:P],
                    warm_sb[:],
                    start=True,
                    stop=True,
                    perf_mode=DR,
                )

            # Startup DMA: sync carries inputs in priority order; scalar
            # carries wb0/wb1 concurrently.
            w8_dma(0)
            nc.sync.dma_start(x8_sb[:, 0, : KO // 2, :], xt8[0, :, : KO // 2, :])
            for i in range(1, HOIST):
                w8_dma(i)
            nc.sync.dma_start(x8_sb[:, 0, KO // 2 :, :], xt8[0, :, KO // 2 :, :])
            nc.sync.dma_start(bias_sb[:], bs[:])
            wb_dma(0)
            wb_dma(1)
            first0 = _chunk_runs(ksb[perm[0]], 2, XR_CHUNK)
            cov0 = {k for a, b in first0 for k in range(a, b)}
            rest0 = _chunk_runs([k for k in range(KO) if k not in cov0], 0, XR_CHUNK)
            xr_stream(0, first0)
            nc.sync.dma_start(x8_sb[:, 1, : KO // 2, :], xt8[1, :, : KO // 2, :])
            xr_stream(0, rest0)
            nc.sync.dma_start(x8_sb[:, 1, KO // 2 :, :], xt8[1, :, KO // 2 :, :])
            xr_stream(1, first0)
            xr_stream(1, rest0)

            def dr_pairs(oc):
                ks = ks8[oc]
                return [(ks[2 * j], ks[2 * j + 1]) for j in range(ns[oc] // 2)]

            def dr_mm(i, psv, win, lo2, width, oc, j2, start, stop):
                k1, k2 = ks8[oc][2 * j2], ks8[oc][2 * j2 + 1]
                nc.tensor.matmul(
                    psv,
                    w8_tiles[i][:, 2 * j2 : 2 * j2 + 2, :],
                    x8_sb[:, win, k1 : k2 + 1 : (k2 - k1), lo2 : lo2 + width],
                    start=start,
                    stop=stop,
                    perf_mode=DR,
                )

            def bf_mms(i, psv, win, lo2, width, oc, start, stop):
                wbt = wb_tiles.get(i)
                m = KO - ns[oc]
                for j in range(m):
                    k = ksb[oc][j]
                    nc.tensor.matmul(
                        psv,
                        wbt[:, j, :],
                        xb_sb[:, win, k, lo2 : lo2 + width],
                        start=(start and j == 0),
                        stop=(stop and j == m - 1),
                    )

            # Hoisted DR groups: out-tiles 0..HOIST-1, window 0, on PSUM
            # banks 1..7 (bank 0 is the warmup's, freed immediately).
            # Low-k pairs for all tiles first so the PE doesn't wait on the
            # second half of x8 window 0.
            ps_handles = {}
            for i in range(HOIST):
                ps_handles[(i, 0)] = pspool.tile(
                    [P, T_FREE], f32, tag="ps", name=f"ps_{i}_0"
                )
            started = set()
            for phase in (0, 1):
                for i in range(HOIST):
                    oc = perm[i]
                    for j2, (k1, k2) in enumerate(dr_pairs(oc)):
                        if (k2 < KO // 2) == (phase == 0):
                            first = i not in started
                            if first:
                                started.add(i)
                            dr_mm(
                                i,
                                ps_handles[(i, 0)][:],
                                0,
                                0,
                                T_FREE,
                                oc,
                                j2,
                                start=first,
                                stop=False,
                            )

            def evict(i, oc, psv, lo, width, split):
                o_sb = opool.tile([P, T_FREE], f32, tag="o", name=f"o_{i}_{lo}")
                nc.scalar.activation(
                    o_sb[:, 0:width],
                    psv,
                    mybir.ActivationFunctionType.Identity,
                    bias=bias_sb[:, oc : oc + 1],
                    scale=S_OUT,
                )
                if not split:
                    nc.scalar.dma_start(
                        yt[oc, :, lo : lo + width], o_sb[:, 0:width]
                    )
                else:
                    pq = P // 2
                    for q in range(2):
                        eng = nc.sync if q == 1 else nc.scalar
                        eng.dma_start(
                            yt[oc, q * pq : (q + 1) * pq, lo : lo + width],
                            o_sb[q * pq : (q + 1) * pq, 0:width],
                        )
                prefetch_after_evict()

            def process(i, win):
                oc = perm[i]
                last = i == OC - 1
                if last and win == 1:
                    windows = [(0, 256), (256, 256)]
                else:
                    windows = [(0, T_FREE)]
                for lo2, width in windows:
                    ps = ps_handles.pop((i, win), None)
                    hoisted = ps is not None
                    if not hoisted:
                        ps = pspool.tile(
                            [P, T_FREE], f32, tag="ps", name=f"ps_{i}_{win}_{lo2}"
                        )
                    psv = ps[:, 0:width]
                    n = ns[oc]
                    if hoisted:
                        # group already started with its DRs; close with bf16
                        bf_mms(i, psv, win, lo2, width, oc, start=(n == 0), stop=True)
                    else:
                        # bf16 first (cheap weight load at the window
                        # boundary), fp8 DR pairs last
                        bf_mms(
                            i, psv, win, lo2, width, oc, start=True, stop=(n == 0)
                        )
                        for j2 in range(n // 2):
                            dr_mm(
                                i,
                                psv,
                                win,
                                lo2,
                                width,
                                oc,
                                j2,
                                start=False,
                                stop=(j2 == n // 2 - 1),
                            )
                    lo = win * T_FREE + lo2
                    evict(i, oc, psv, lo, width, split=last)

            # Phase 1: close the hoisted window-0 groups with bf16.
            for i in range(HOIST):
                process(i, 0)
            # Phase 2: the hoisted out-tiles' window 1 (w8 prefetch opens).
            w8_gate[0] = True
            for i in range(HOIST):
                process(i, 1)
            # Phase 3: remaining out-tiles, both windows.
            for i in range(HOIST, OC):
                process(i, 0)
                process(i, 1)

    nc.compile()
    return nc


def _pack_inputs(x, weight, bias, plan, x8, xr8):
    import ml_dtypes

    E4 = ml_dtypes.float8_e4m3
    BF = ml_dtypes.bfloat16

    w8 = (weight * SW).astype(E4).reshape(OC, P, KO, P)
    wb = (weight * SW).astype(BF).reshape(OC, P, KO, P)

    xt8 = np.ascontiguousarray(
        x8.reshape(N_CORES, NT, T_FREE, KO, P).transpose(0, 1, 4, 3, 2)
    )
    xtr = np.ascontiguousarray(
        xr8.reshape(N_CORES, NT, T_FREE, KO, P).transpose(0, 1, 4, 3, 2)
    )
    wp8 = np.ascontiguousarray(
        np.concatenate(
            [w8[oc][:, plan["ks8"][oc], :].transpose(2, 1, 0) for oc in range(OC)],
            axis=1,
        )
    )
    wpb = np.ascontiguousarray(
        np.concatenate(
            [wb[oc][:, plan["ksb"][oc], :].transpose(2, 1, 0) for oc in range(OC)],
            axis=1,
        )
    )
    bsr = np.ascontiguousarray(bias.reshape(OC, P).T)
    return xt8, xtr, wp8, wpb, bsr


def kernel(x, weight, bias):
    global LAST_EXEC_NS
    from concourse import bass_utils

    x = np.ascontiguousarray(x, dtype=np.float32)
    weight = np.ascontiguousarray(weight, dtype=np.float32)
    bias = np.ascontiguousarray(bias, dtype=np.float32)

    x8, xr8, x8d, xbd = _quant_arrays(x)

    if "nc" not in _cache:
        sel = _select_tiles(x, weight, x8d, xbd)
        plan = _make_plan(sel)
        _cache["plan"] = plan
        _cache["nc"] = _build_bass(plan)
    nc = _cache["nc"]
    plan = _cache["plan"]

    xt8, xtr, wp8, wpb, bsr = _pack_inputs(x, weight, bias, plan, x8, xr8)

    in_maps = [
        {"xt8": xt8[c], "xtr": xtr[c], "wp8": wp8, "wpb": wpb, "bs": bsr}
        for c in range(N_CORES)
    ]

    trace = bool(int(os.environ.get("BSL_TRACE", "0")))
    kw = {}
    if os.environ.get("BSL_TMPDIR"):
        kw["tmpdir"] = os.environ["BSL_TMPDIR"]
    res = bass_utils.run_bass_kernel_spmd(
        nc,
        in_maps,
        core_ids=list(range(N_CORES)),
        trace=trace,
        **kw,
    )
    _cache["res"] = res
    LAST_EXEC_NS = res.exec_time_ns

    out = np.empty((TOK, D_OUT), dtype=np.float32)
    for c in range(N_CORES):
        yt_out = res.results[c]["yt"]
        out[c * T_PER_CORE : (c + 1) * T_PER_CORE] = (
            yt_out.transpose(2, 0, 1).reshape(T_PER_CORE, D_OUT)
        )
    return out
